# revision 1
# baseline (speedup 1.0000x reference)
"""MMD loss (RBF kernel) on 8 Trainium2 NeuronCores.

Contract: kernel(input, target, sigma) -> np.float32 scalar (full inputs in,
full output out; sharding is internal).

Math: result = mean(XX) + mean(YY) - 2*mean(XY), where e.g.
  XX[i,j] = exp(-||x_i-x_j||^2/sigma) = exp(2*x_i.x_j/sigma - x2_i/sigma - x2_j/sigma)

Sharding: core c owns a 512-row block (i) of each of the three 4096x4096
grams.  Per gram the device computes, in transposed tile layout
[j=128 partitions, i=512 free]:
  A[j,i] = exp((2/sigma)*g_ij + bias_j),   g = <row_j, row_i> via PE matmul
with the column-norm term and a per-core shift C folded into the Exp
activation's per-partition bias (so A <= 1, no overflow for any sigma).
A ones-vector matmul then reduces over j into a [1,512] PSUM accumulator
across all 32 j-chunks.  The remaining per-row factor exp(C - x2_i/sigma)
factors out of the j-sum and is applied on host, which also combines the
8 cores' partial sums.  For tiny sigma (<32) a host fallback avoids
underflow pathologies entirely.
"""

import numpy as np
import ml_dtypes

N = 4096
D = 256
NCORES = 8
BLK = N // NCORES  # 512
NJ = N // 128      # 32 j-chunks per gram


def _build(scale: float):
    """Raw-bass SPMD kernel (one NeuronCore's program; data differs per core).

    Engine pipeline, idx = g*32+m over 3 grams x 32 j-chunks:
      PE : 2 accumulating matmuls -> p[idx%4] (PSUM, [128j,512i] = gram block),
           plus, lagging 2 behind, a ones-matmul reducing a[j%6] over
           partitions into racc [1,512] (accumulated over the gram's 32 chunks)
      ACT: a[idx%6] = exp(scale*p + bias_j) (per-partition bias from btile)
      DVE: after each gram, copy racc -> out_sb slice
      SP : input DMAs up front, output DMA at the end
    Raw bass (not Tile): this container's walrus rejects >1 embedded
    sync-wait per instruction, which Tile's scheduler and tail drain emit.
    """
    import concourse.bass as bass
    from concourse import mybir

    bf16 = mybir.dt.bfloat16
    f32 = mybir.dt.float32

    NIDX = 3 * NJ           # 96 pipeline steps
    NP = 4                  # p (PSUM) buffers
    NA = 6                  # a (SBUF) buffers
    LAG = 2                 # ones-matmul runs LAG behind the main matmuls

    nc = bass.Bass()
    xt_d = nc.declare_dram_parameter("xt", [2, 128, N], bf16, isOutput=False)
    yt_d = nc.declare_dram_parameter("yt", [2, 128, N], bf16, isOutput=False)
    xbt_d = nc.declare_dram_parameter("xbt", [2, 128, BLK], bf16, isOutput=False)
    ybt_d = nc.declare_dram_parameter("ybt", [2, 128, BLK], bf16, isOutput=False)
    bias_d = nc.declare_dram_parameter("bias", [128, 3 * NJ], f32, isOutput=False)
    ones_d = nc.declare_dram_parameter("ones", [128, 1], bf16, isOutput=False)
    out_d = nc.declare_dram_parameter("out", [1, 3 * BLK], f32, isOutput=True)

    from contextlib import ExitStack
    with ExitStack() as ctx:
        xt0 = ctx.enter_context(nc.sbuf_tensor([128, N], bf16))
        xt1 = ctx.enter_context(nc.sbuf_tensor([128, N], bf16))
        yt0 = ctx.enter_context(nc.sbuf_tensor([128, N], bf16))
        yt1 = ctx.enter_context(nc.sbuf_tensor([128, N], bf16))
        xbt0 = ctx.enter_context(nc.sbuf_tensor([128, BLK], bf16))
        xbt1 = ctx.enter_context(nc.sbuf_tensor([128, BLK], bf16))
        ybt0 = ctx.enter_context(nc.sbuf_tensor([128, BLK], bf16))
        ybt1 = ctx.enter_context(nc.sbuf_tensor([128, BLK], bf16))
        btile = ctx.enter_context(nc.sbuf_tensor([128, 3 * NJ], f32))
        ones = ctx.enter_context(nc.sbuf_tensor([128, 1], bf16))
        out_sb = ctx.enter_context(nc.sbuf_tensor([1, 3 * BLK], f32))
        ps = [ctx.enter_context(nc.psum_tensor(f"p{i}", [128, BLK], f32))
              for i in range(NP)]
        raccs = [ctx.enter_context(nc.psum_tensor(f"racc{g}", [1, BLK], f32))
                 for g in range(3)]
        avs = [ctx.enter_context(nc.sbuf_tensor(f"a{i}", [128, BLK], bf16))
               for i in range(NA)]
        dma_sem = ctx.enter_context(nc.semaphore("dma_sem"))
        pe_sem = ctx.enter_context(nc.semaphore("pe_sem"))
        pe2_sem = ctx.enter_context(nc.semaphore("pe2_sem"))
        act_sem = ctx.enter_context(nc.semaphore("act_sem"))
        cp_sem = ctx.enter_context(nc.semaphore("cp_sem"))
        block = ctx.enter_context(nc.Block())

        NDMA_CH = 8  # DMA chunks per big matrix tile
        CH = N // NDMA_CH
        n_loads = 4 * NDMA_CH + 4 + 2  # big tiles + block tiles + bias + ones

        grams = [
            ((xt0, xt1), (xbt0, xbt1)),  # XX: j over X rows, i over X block
            ((yt0, yt1), (ybt0, ybt1)),  # YY: j over Y rows, i over Y block
            ((yt0, yt1), (xbt0, xbt1)),  # XY: j over Y rows, i over X block
        ]

        def ones_mm(tensor, j):
            # each gram accumulates into its own PSUM bank, so PE never
            # waits on DVE's result copies
            gj, mj = divmod(j, NJ)
            tensor.wait_ge(act_sem, j + 1)
            tensor.matmul(raccs[gj][:], ones[:], avs[j % NA][:],
                          start=(mj == 0), stop=(mj == NJ - 1),
                          ).then_inc(pe2_sem, 1)

        # batch 1: everything the XX gram (and ACT bias) needs — 20 loads;
        # batch 2 (Y side) is issued only after PE's first matmul completes,
        # so PE's `dma_sem >= 16*N_B1` wait unambiguously means batch 1 is
        # done (completion order across DMA queues is otherwise unordered).
        N_B1 = 4 + 2 * NDMA_CH

        @block.sync
        def _(sync):
            sync.dma_start(xbt0[:], xbt_d[0]).then_inc(dma_sem, 16)
            sync.dma_start(xbt1[:], xbt_d[1]).then_inc(dma_sem, 16)
            sync.dma_start(btile[:], bias_d[:]).then_inc(dma_sem, 16)
            sync.dma_start(ones[:], ones_d[:]).then_inc(dma_sem, 16)
            for q in range(NDMA_CH):
                for t, src in ((xt0, xt_d[0]), (xt1, xt_d[1])):
                    sync.dma_start(t[:, bass.ts(q, CH)],
                                   src[:, bass.ts(q, CH)]).then_inc(dma_sem, 16)
            sync.wait_ge(pe_sem, 1)
            sync.dma_start(ybt0[:], ybt_d[0]).then_inc(dma_sem, 16)
            sync.dma_start(ybt1[:], ybt_d[1]).then_inc(dma_sem, 16)
            for q in range(NDMA_CH):
                for t, src in ((yt0, yt_d[0]), (yt1, yt_d[1])):
                    sync.dma_start(t[:, bass.ts(q, CH)],
                                   src[:, bass.ts(q, CH)]).then_inc(dma_sem, 16)
            sync.wait_ge(cp_sem, 3)
            sync.dma_start(out_d[:], out_sb[:]).then_inc(dma_sem, 16)

        @block.tensor
        def _(tensor):
            tensor.wait_ge(dma_sem, 16 * N_B1)
            for idx in range(NIDX):
                g, m = divmod(idx, NJ)
                if idx == NJ:
                    # Y-side operands (batch 2) must be resident for YY/XY
                    tensor.wait_ge(dma_sem, 16 * n_loads)
                (l0, l1), (r0, r1) = grams[g]
                if idx >= NP:
                    # p-slot reuse: ACT must have consumed p[idx-NP]
                    tensor.wait_ge(act_sem, idx - NP + 1)
                tensor.matmul(ps[idx % NP][:], l0[:, bass.ts(m, 128)], r0[:],
                              start=True, stop=False)
                tensor.matmul(ps[idx % NP][:], l1[:, bass.ts(m, 128)], r1[:],
                              start=False, stop=True).then_inc(pe_sem, 1)
                if idx >= LAG:
                    ones_mm(tensor, idx - LAG)
            for j in range(NIDX - LAG, NIDX):
                ones_mm(tensor, j)

        @block.scalar
        def _(scalar):
            for idx in range(NIDX):
                scalar.wait_ge(pe_sem, idx + 1)
                if idx >= NA:
                    # a-slot reuse: PE ones-matmul must have consumed a[idx-NA]
                    scalar.wait_ge(pe2_sem, idx - NA + 1)
                scalar.activation(
                    avs[idx % NA][:], ps[idx % NP][:],
                    mybir.ActivationFunctionType.Exp,
                    bias=btile[:, idx : idx + 1], scale=scale,
                ).then_inc(act_sem, 1)

        @block.vector
        def _(vector):
            for g in range(3):
                vector.wait_ge(pe2_sem, NJ * (g + 1))
                vector.tensor_copy(out_sb[:, g * BLK : (g + 1) * BLK],
                                   raccs[g][:]).then_inc(cp_sem, 1)

    return nc


def _prepare(x, y, sigma):
    bf16 = ml_dtypes.bfloat16
    x64 = x.astype(np.float64)
    y64 = y.astype(np.float64)
    x2 = (x64 * x64).sum(1)  # [N]
    y2 = (y64 * y64).sum(1)
    xt = np.ascontiguousarray(x.T).reshape(2, 128, N).astype(bf16)
    yt = np.ascontiguousarray(y.T).reshape(2, 128, N).astype(bf16)
    in_maps = []
    posts = []
    for c in range(NCORES):
        sl = slice(c * BLK, (c + 1) * BLK)
        xbt = np.ascontiguousarray(x.T[:, sl]).reshape(2, 128, BLK).astype(bf16)
        ybt = np.ascontiguousarray(y.T[:, sl]).reshape(2, 128, BLK).astype(bf16)
        cx = float(x2[sl].max() / sigma)
        cy = float(y2[sl].max() / sigma)
        bias = np.concatenate([
            (-x2 / sigma - cx).reshape(NJ, 128).T,
            (-y2 / sigma - cy).reshape(NJ, 128).T,
            (-y2 / sigma - cx).reshape(NJ, 128).T,
        ], axis=1).astype(np.float32)
        ux = np.exp(cx - x2[sl] / sigma)
        uy = np.exp(cy - y2[sl] / sigma)
        in_maps.append({
            "xt": xt, "yt": yt,
            "xbt": xbt, "ybt": ybt,
            "bias": np.ascontiguousarray(bias),
            "ones": np.ones((128, 1), dtype=bf16),
        })
        posts.append((ux, uy))
    return in_maps, posts


def _host_reference(x, y, sigma):
    x = x.astype(np.float64)
    y = y.astype(np.float64)

    def s(a, b):
        a2 = (a * a).sum(1)
        b2 = (b * b).sum(1)
        tot = 0.0
        for i0 in range(0, a.shape[0], 512):
            d2 = a2[i0:i0 + 512, None] + b2[None, :] - 2.0 * (a[i0:i0 + 512] @ b.T)
            np.maximum(d2, 0.0, out=d2)
            tot += float(np.exp(-d2 / sigma).sum())
        return tot

    n = x.shape[0]
    m = y.shape[0]
    return np.float32(s(x, x) / (n * n) + s(y, y) / (m * m) - 2.0 * s(x, y) / (n * m))


def _run(input, target, sigma, trace=False):
    sig = float(np.asarray(sigma))
    x = np.asarray(input, np.float32)
    y = np.asarray(target, np.float32)
    if sig < 32.0:
        return _host_reference(x, y, sig), None
    from concourse.bass_utils import run_bass_kernel_spmd
    in_maps, posts = _prepare(x, y, sig)
    nc = _build(2.0 / sig)
    try:
        bkr = run_bass_kernel_spmd(nc, in_maps, list(range(NCORES)), trace=trace)
    except (ImportError, ModuleNotFoundError):
        # NTFF profile hook unavailable in this container; run untraced.
        bkr = run_bass_kernel_spmd(nc, in_maps, list(range(NCORES)), trace=False)
    sxx = syy = sxy = 0.0
    for c in range(NCORES):
        r = bkr.results[c]["out"].astype(np.float64).reshape(3, BLK)
        ux, uy = posts[c]
        sxx += float(r[0] @ ux)
        syy += float(r[1] @ uy)
        sxy += float(r[2] @ ux)
    val = (sxx + syy - 2.0 * sxy) / (float(N) * float(N))
    return np.float32(val), bkr


def kernel(input, target, sigma):
    val, _ = _run(input, target, sigma)
    return val



# revision 2
# speedup vs baseline: 15.6902x; 15.6902x over previous
"""MMD loss (RBF kernel) on 8 Trainium2 NeuronCores — pure-JAX shard_map.

Contract: kernel(input, target, sigma) -> np.float32 scalar (full inputs in,
full output out; sharding is internal).

Math: result = mean(XX) + mean(YY) - 2*mean(XY), where e.g.
  XX[i,j] = exp(-max(||x_i||^2 + ||x_j||^2 - 2 x_i.x_j, 0) / sigma)

Why this shape: on this axon-tunneled setup the per-call RPC round trip
(~80-90 ms) and host->device tunnel bandwidth dwarf device compute (<1 ms
for the 25 GFLOP of grams).  So the whole loss is ONE jitted shard_map
program: x and y ship bf16 ROW-SHARDED over the 8 cores (0.5 MB/core
instead of 4.5 MB/core replicated), the full matrices are all-gathered
device-side over NeuronLink, each core computes its 512-row block of the
three grams (bf16 PE matmuls, f32 accumulate) plus row norms, and a psum
folds the partial sums into one replicated f32 scalar — a single small
d2h fetch.  The jitted callable is cached at module level so warm calls
skip trace/compile entirely.
"""

import numpy as np
import ml_dtypes

N = 4096
D = 256
NCORES = 8
BLK = N // NCORES  # 512

_FN = None


def _get_fn():
    global _FN
    if _FN is not None:
        return _FN
    import jax
    import jax.numpy as jnp
    from jax.sharding import Mesh, PartitionSpec as P

    try:
        from jax import shard_map

        def _smap(f, mesh, in_specs, out_specs):
            return shard_map(
                f, mesh=mesh, in_specs=in_specs, out_specs=out_specs, check_vma=False
            )
    except ImportError:
        from jax.experimental.shard_map import shard_map

        def _smap(f, mesh, in_specs, out_specs):
            return shard_map(
                f, mesh=mesh, in_specs=in_specs, out_specs=out_specs, check_rep=False
            )

    devices = jax.devices()[:NCORES]
    mesh = Mesh(np.asarray(devices), ("core",))

    def _body(xb, yb, sigma):
        # xb, yb: [BLK, D] bf16 (this core's row block); sigma: f32 scalar
        xf = jax.lax.all_gather(xb, "core", tiled=True)  # [N, D] bf16
        yf = jax.lax.all_gather(yb, "core", tiled=True)
        x2b = jnp.sum(xb.astype(jnp.float32) ** 2, axis=1)  # [BLK]
        y2b = jnp.sum(yb.astype(jnp.float32) ** 2, axis=1)
        x2f = jax.lax.all_gather(x2b, "core", tiled=True)  # [N]
        y2f = jax.lax.all_gather(y2b, "core", tiled=True)

        def gram_sum(ab, a2b, bf, b2f):
            g = jnp.matmul(ab, bf.T, preferred_element_type=jnp.float32)
            d2 = a2b[:, None] + b2f[None, :] - 2.0 * g
            d2 = jnp.maximum(d2, 0.0)
            return jnp.sum(jnp.exp(-d2 / sigma))

        sxx = gram_sum(xb, x2b, xf, x2f)
        syy = gram_sum(yb, y2b, yf, y2f)
        sxy = gram_sum(xb, x2b, yf, y2f)
        part = sxx + syy - 2.0 * sxy
        tot = jax.lax.psum(part, "core")
        return tot / (float(N) * float(N))

    _FN = jax.jit(
        _smap(_body, mesh=mesh, in_specs=(P("core"), P("core"), P()), out_specs=P())
    )
    return _FN


def kernel(input, target, sigma):
    x = np.asarray(input, dtype=np.float32)
    y = np.asarray(target, dtype=np.float32)
    sig = np.float32(np.asarray(sigma))
    bf16 = ml_dtypes.bfloat16
    xb = np.ascontiguousarray(x).astype(bf16)
    yb = np.ascontiguousarray(y).astype(bf16)
    fn = _get_fn()
    out = fn(xb, yb, sig)
    return np.float32(np.asarray(out))
